# revision 1
# baseline (speedup 1.0000x reference)
"""GCNConv Trainium2 kernel: 8-core SPMD via bass/Tile.

Strategy (dst-range edge sharding; one shared SPMD program, all data per-core):
  - core c owns dst nodes [c*NSH, (c+1)*NSH) and all edges into them
  - x~ = feat @ fc_w.T + edge_b table built on device. Each core's table is
    ROTATED so its own node range sits at rows [0, NSH) (keeps the program
    core-independent; the rotation is folded into the host-built gather indices)
  - per-edge: dma_gather x~[src] (512B rows; 4 src-range buckets for int16 idx),
    w = edge_feat @ edge_w.T on PE (bf16 in, fp32 acc), m = relu(dis_src*(x~+w))
    on ACT, one-hot(dst) via DVE/GPSIMD tensor_scalar, segment-sum via fp32r
    matmul into PSUM h^T [feat, nodes] super-windows at register-offset columns
  - node side: out = h*dis + relu(x+root)/deg, via PE transpose of h^T
"""
import sys, math
sys.path.insert(0, "/opt/trn_rl_repo")
import numpy as np

from concourse import bass, bacc, mybir, tile
from concourse import bass_utils

f32 = mybir.dt.float32
f32r = mybir.dt.float32r
bf16 = mybir.dt.bfloat16
i16 = mybir.dt.int16
i32 = mybir.dt.int32
RELU = mybir.ActivationFunctionType.Relu
ALU = mybir.AluOpType


class Cfg:
    def __init__(self, N=100000, E=1600000, F=128, ED=7, cores=8,
                 sw_nodes=2048, group=256, cap_full=75, cap_last=12,
                 gather_chunks=25):
        self.N, self.E, self.F, self.ED, self.cores = N, E, F, ED, cores
        assert N % cores == 0
        self.NSH = N // cores                    # nodes per core
        self.SW = sw_nodes                       # super-window width (<=4 psum banks)
        self.GRP = group                         # one-hot width / matmul N
        self.n_sw = math.ceil(self.NSH / sw_nodes)
        self.last_w = self.NSH - (self.n_sw - 1) * sw_nodes
        self.cap = [cap_full] * (self.n_sw - 1) + [cap_last]
        self.gb = gather_chunks                  # max chunks per dma_gather call
        self.n_buckets = 4
        self.bucket_sz = math.ceil(N / self.n_buckets)
        assert self.bucket_sz <= 32768
        self.n_chunks = self.n_buckets * sum(self.cap)
        self.slots = self.n_chunks * 128
        self.n_tiles = math.ceil(N / 128)        # x~ build tiles
        self.Npad = self.n_tiles * 128
        self.nsh_tiles = math.ceil(self.NSH / 128)
        self.NSHpad = self.nsh_tiles * 128

    def call_layout(self):
        """Gather-call boundaries [(slot_start, chunks_in_call)] in schedule order."""
        out = []
        si = 0
        for s in range(self.n_sw):
            for _b in range(self.n_buckets):
                rem = self.cap[s]
                while rem > 0:
                    n = min(self.gb, rem)
                    out.append((si, n))
                    si += n * 128
                    rem -= n
        assert si == self.slots
        return out


CFG = Cfg()
_PROG_CACHE = {}


# ---------------------------------------------------------------- program ----
def build_program(cfg: Cfg):
    nc = bacc.Bacc("TRN2", target_bir_lowering=False, debug=False,
                   num_devices=cfg.cores)
    F, GRP, SW = cfg.F, cfg.GRP, cfg.SW

    featT_d = nc.dram_tensor("featT", [F, cfg.Npad], f32, kind="ExternalInput")
    fcwT_d = nc.dram_tensor("fcwT", [F, F], f32, kind="ExternalInput")
    ewT8_d = nc.dram_tensor("ewT8", [8, F], bf16, kind="ExternalInput")
    edgebB_d = nc.dram_tensor("edgebB", [128, F], f32, kind="ExternalInput")
    rootB_d = nc.dram_tensor("rootB", [128, F], f32, kind="ExternalInput")
    iota_d = nc.dram_tensor("iotaG", [128, GRP], f32, kind="ExternalInput")
    ident_d = nc.dram_tensor("ident", [128, 128], f32, kind="ExternalInput")
    efT_d = nc.dram_tensor("efT", [8, cfg.slots], bf16, kind="ExternalInput")
    idx_d = nc.dram_tensor("idxw", [128, cfg.slots // 16], i16, kind="ExternalInput")
    rel_d = nc.dram_tensor("relT", [128, cfg.n_chunks], f32, kind="ExternalInput")
    disS_d = nc.dram_tensor("disS", [128, cfg.n_chunks], f32, kind="ExternalInput")
    goff_d = nc.dram_tensor("goff", [1, cfg.n_chunks], i32, kind="ExternalInput")
    disP_d = nc.dram_tensor("disP", [128, cfg.nsh_tiles], f32, kind="ExternalInput")
    ivdP_d = nc.dram_tensor("ivdP", [128, cfg.nsh_tiles], f32, kind="ExternalInput")

    xt_d = nc.dram_tensor("xtab", [cfg.Npad, F], f32, kind="Internal")
    out_d = nc.dram_tensor("out", [cfg.NSHpad, F], f32, kind="ExternalOutput")

    with tile.TileContext(nc) as tc:
        with tc.tile_pool(name="persist", bufs=1) as pers:
            fcwT = pers.tile([F, F], f32)
            nc.sync.dma_start(out=fcwT[:], in_=fcwT_d.ap())
            ewT8 = pers.tile([8, F], bf16)
            nc.sync.dma_start(out=ewT8[:], in_=ewT8_d.ap())
            edgebB = pers.tile([128, F], f32)
            nc.sync.dma_start(out=edgebB[:], in_=edgebB_d.ap())
            rootB = pers.tile([128, F], f32)
            nc.sync.dma_start(out=rootB[:], in_=rootB_d.ap())
            iotaG = pers.tile([128, GRP], f32)
            nc.sync.dma_start(out=iotaG[:], in_=iota_d.ap())
            ident = pers.tile([128, 128], f32)
            nc.sync.dma_start(out=ident[:], in_=ident_d.ap())
            zero128 = pers.tile([128, 128], bf16)
            nc.vector.memset(zero128[:], 0.0)
            zero512 = pers.tile([128, 512], bf16)
            nc.vector.memset(zero512[:], 0.0)
            relT = pers.tile([128, cfg.n_chunks], f32)
            nc.sync.dma_start(out=relT[:], in_=rel_d.ap())
            disS = pers.tile([128, cfg.n_chunks], f32)
            nc.sync.dma_start(out=disS[:], in_=disS_d.ap())
            goffT = pers.tile([1, cfg.n_chunks], i32)
            nc.sync.dma_start(out=goffT[:], in_=goff_d.ap())
            idxw = pers.tile([128, cfg.slots // 16], i16)
            nc.sync.dma_start(out=idxw[:], in_=idx_d.ap())
            disP = pers.tile([128, cfg.nsh_tiles], f32)
            nc.sync.dma_start(out=disP[:], in_=disP_d.ap())
            ivdP = pers.tile([128, cfg.nsh_tiles], f32)
            nc.sync.dma_start(out=ivdP[:], in_=ivdP_d.ap())
            hT = pers.tile([128, cfg.NSHpad], f32)   # h^T accumulator [feat, node]
            nc.vector.memset(hT[:], 0.0)

            # ================= phase 1: x~ table =================
            with (
                tc.tile_pool(name="xph", bufs=3) as xph,
                tc.tile_pool(name="xps", bufs=4, space="PSUM") as xps,
            ):
                BLK = 8
                nblk = math.ceil(cfg.n_tiles / BLK)
                for blk in range(nblk):
                    t0 = blk * BLK
                    nt = min(BLK, cfg.n_tiles - t0)
                    ft = xph.tile([F, BLK * 128], f32, tag="ft")
                    nc.sync.dma_start(
                        out=ft[:, :nt * 128],
                        in_=featT_d.ap()[:, t0 * 128:(t0 + nt) * 128])
                    xt = xph.tile([128, BLK, F], f32, tag="xt")
                    for j in range(nt):
                        px = xps.tile([128, F], f32, tag="px")
                        nc.tensor.matmul(out=px[:], lhsT=ft[:, j * 128:(j + 1) * 128],
                                         rhs=fcwT[:], start=True, stop=True)
                        nc.vector.tensor_add(out=xt[:, j, :], in0=px[:], in1=edgebB[:])
                    nc.sync.dma_start(
                        out=xt_d.ap()[t0 * 128:(t0 + nt) * 128, :].rearrange(
                            "(b p) f -> p b f", p=128),
                        in_=xt[:, :nt, :])

            # ================= phase 2: edges =================
            with (
                tc.tile_pool(name="eph", bufs=2) as eph,
                tc.tile_pool(name="mph", bufs=4) as mph,
                tc.tile_pool(name="hps_pool", bufs=1, space="PSUM") as hps_pool,
                tc.tile_pool(name="wps_pool", bufs=4, space="PSUM") as wps_pool,
            ):
                hps = hps_pool.tile([128, SW], f32)
                ci = 0
                si = 0
                for sw in range(cfg.n_sw):
                    for bank in range(SW // 512):
                        nc.tensor.matmul(
                            out=hps[:, bank * 512:(bank + 1) * 512],
                            lhsT=zero128[:],
                            rhs=zero512[:],
                            start=True, stop=False, skip_group_check=True)
                    for b in range(cfg.n_buckets):
                        base = b * cfg.bucket_sz
                        bucket_ap = xt_d.ap()[base:min(base + cfg.bucket_sz,
                                                       cfg.Npad), :]
                        rem = cfg.cap[sw]
                        call_sizes = []
                        while rem > 0:
                            call_sizes.append(min(cfg.gb, rem))
                            rem -= call_sizes[-1]
                        for ncall in call_sizes:
                            nidx = ncall * 128
                            gout = eph.tile([128, cfg.gb, F], f32, tag="gout")
                            nc.gpsimd.dma_gather(
                                out_ap=gout[:, :ncall, :],
                                in_ap=bucket_ap,
                                idxs_ap=idxw[:, si // 16:(si + nidx) // 16],
                                num_idxs=nidx, num_idxs_reg=nidx, elem_size=F,
                                single_packet=False)
                            ef = eph.tile([8, cfg.gb * 128], bf16, tag="ef")
                            nc.sync.dma_start(
                                out=ef[:, :nidx], in_=efT_d.ap()[:, si:si + nidx])
                            for kk in range(ncall):
                                pw = wps_pool.tile([128, F], f32, tag="pw")
                                nc.tensor.matmul(
                                    out=pw[:], lhsT=ef[:, kk * 128:(kk + 1) * 128],
                                    rhs=ewT8[:], start=True, stop=True)
                                mpre = mph.tile([128, F], f32, tag="mpre")
                                nc.vector.tensor_add(
                                    out=mpre[:], in0=gout[:, kk, :], in1=pw[:])
                                m = mph.tile([128, F], f32r, tag="m")
                                nc.scalar.activation(
                                    out=m[:], in_=mpre[:], func=RELU,
                                    scale=disS[:, ci:ci + 1])
                                oh = mph.tile([128, GRP], f32r, tag="oh")
                                eng = nc.vector if (ci % 2 == 0) else nc.gpsimd
                                eng.tensor_scalar(
                                    out=oh[:], in0=iotaG[:],
                                    scalar1=relT[:, ci:ci + 1], op0=ALU.subtract,
                                    scalar2=0.0, op1=ALU.is_equal)
                                with tc.tile_critical():
                                    reg = nc.tensor.alloc_register(f"go{ci}")
                                    nc.tensor.reg_load(reg, goffT[0:1, ci:ci + 1])
                                    val = nc.snap(reg, donate=True, min_val=0,
                                                  max_val=SW - GRP)
                                    nc.tensor.matmul(
                                        out=hps[:, bass.ds(val, GRP)],
                                        lhsT=m[:],
                                        rhs=oh[:],
                                        start=False, stop=False,
                                        skip_group_check=True)
                                ci += 1
                                si += 128
                    w = SW if sw < cfg.n_sw - 1 else cfg.last_w
                    nc.vector.tensor_add(
                        out=hT[:, sw * SW:sw * SW + w],
                        in0=hT[:, sw * SW:sw * SW + w], in1=hps[:, :w])
                assert ci == cfg.n_chunks and si == cfg.slots

            # ================= phase 3: node-side =================
            with (
                tc.tile_pool(name="nph", bufs=3) as nph,
                tc.tile_pool(name="nps", bufs=4, space="PSUM") as nps,
            ):
                NBLK = 8
                for blk in range(math.ceil(cfg.nsh_tiles / NBLK)):
                    t0 = blk * NBLK
                    nt = min(NBLK, cfg.nsh_tiles - t0)
                    xtile = nph.tile([128, NBLK, F], f32, tag="xtile")
                    nc.sync.dma_start(
                        out=xtile[:, :nt, :],
                        in_=xt_d.ap()[t0 * 128:(t0 + nt) * 128, :].rearrange(
                            "(b p) f -> p b f", p=128))
                    ot = nph.tile([128, NBLK, F], f32, tag="ot")
                    for j in range(nt):
                        t = t0 + j
                        pt = nps.tile([128, F], f32, tag="pt")
                        nc.tensor.transpose(
                            out=pt[:], in_=hT[:, t * 128:(t + 1) * 128],
                            identity=ident[:])
                        s1 = nph.tile([128, F], f32, tag="s1")
                        nc.vector.tensor_scalar_mul(
                            out=s1[:], in0=pt[:], scalar1=disP[:, t:t + 1])
                        t1 = nph.tile([128, F], f32, tag="t1")
                        nc.vector.tensor_add(
                            out=t1[:], in0=xtile[:, j, :], in1=rootB[:])
                        s2 = nph.tile([128, F], f32, tag="s2")
                        nc.scalar.activation(
                            out=s2[:], in_=t1[:], func=RELU,
                            scale=ivdP[:, t:t + 1])
                        nc.vector.tensor_add(out=ot[:, j, :], in0=s1[:], in1=s2[:])
                    nc.sync.dma_start(
                        out=out_d.ap()[t0 * 128:(t0 + nt) * 128, :].rearrange(
                            "(b p) f -> p b f", p=128),
                        in_=ot[:, :nt, :])
    nc.compile()
    return nc


# ------------------------------------------------------------- host prep ----
def host_prep(cfg: Cfg, feat, edge_feat, src, dst, fc_w, edge_w, edge_b,
              root_emb):
    N, E, F = cfg.N, cfg.E, cfg.F
    deg = (np.bincount(dst, minlength=N) + 1.0).astype(np.float32)
    dis = deg ** -0.5

    featT_full = np.ascontiguousarray(feat.T).astype(np.float32)   # [F, N]
    fcwT = np.ascontiguousarray(fc_w.T).astype(np.float32)
    ewT8 = np.zeros((8, F), dtype=np.float32)
    ewT8[:cfg.ED] = edge_w.T
    edgebB = np.tile(edge_b[None, :], (128, 1)).astype(np.float32)
    rootB = np.tile((root_emb[0] - edge_b)[None, :], (128, 1)).astype(np.float32)
    iotaG = np.tile(np.arange(cfg.GRP, dtype=np.float32), (128, 1))
    ident = np.eye(128, dtype=np.float32)

    core_of = dst // cfg.NSH
    in_maps = []
    for c in range(cfg.cores):
        sel = np.nonzero(core_of == c)[0]
        # rotated node space: node v -> row (v - c*NSH) mod N
        rsrc = (src[sel] - c * cfg.NSH) % N
        ed = dst[sel] - c * cfg.NSH
        eb = rsrc // cfg.bucket_sz
        sw = ed // cfg.SW
        order = np.lexsort((ed, eb, sw))
        es, ed, eb, sw = rsrc[order], ed[order], eb[order], sw[order]
        eid = sel[order]

        slot_src = np.zeros(cfg.slots, dtype=np.int16)
        slot_rel = np.full(cfg.slots, -1.0, dtype=np.float32)
        slot_dis = np.zeros(cfg.slots, dtype=np.float32)
        slot_eid = np.full(cfg.slots, -1, dtype=np.int64)
        chunk_goff = np.zeros(cfg.n_chunks, dtype=np.int32)

        ci = 0
        comp = sw * cfg.n_buckets + eb
        seg_starts = np.searchsorted(comp, np.arange(cfg.n_sw * cfg.n_buckets + 1))
        for s in range(cfg.n_sw):
            for b in range(cfg.n_buckets):
                lo = seg_starts[s * cfg.n_buckets + b]
                hi = seg_starts[s * cfg.n_buckets + b + 1]
                seg_ed, seg_es, seg_eid = ed[lo:hi], es[lo:hi], eid[lo:hi]
                grp = (seg_ed - s * cfg.SW) // cfg.GRP
                cstart = ci
                gi = 0
                nseg = len(seg_ed)
                while gi < nseg:
                    gj = gi
                    g = int(grp[gi])
                    while gj < nseg and grp[gj] == g:
                        gj += 1
                    for a in range(gi, gj, 128):
                        z = min(a + 128, gj)
                        slot0 = ci * 128
                        n = z - a
                        srcs = seg_es[a:z]
                        slot_src[slot0:slot0 + n] = (
                            srcs - b * cfg.bucket_sz).astype(np.int16)
                        slot_rel[slot0:slot0 + n] = (
                            seg_ed[a:z] - s * cfg.SW - g * cfg.GRP)
                        slot_dis[slot0:slot0 + n] = dis[src[seg_eid[a:z]]]
                        slot_eid[slot0:slot0 + n] = seg_eid[a:z]
                        chunk_goff[ci] = g * cfg.GRP
                        ci += 1
                    gi = gj
                used = ci - cstart
                if used > cfg.cap[s]:
                    raise RuntimeError(
                        f"segment overflow core {c} sw {s} bucket {b}: "
                        f"{used} > {cfg.cap[s]}")
                ci = cstart + cfg.cap[s]
        assert ci == cfg.n_chunks

        real = slot_eid >= 0
        efT = np.zeros((8, cfg.slots), dtype=np.float32)
        efT[:cfg.ED, real] = edge_feat[slot_eid[real]].T
        idxw = np.zeros((16, cfg.slots // 16), dtype=np.int16)
        for s0, nch in cfg.call_layout():
            blkv = slot_src[s0:s0 + nch * 128]
            idxw[:, s0 // 16:(s0 + nch * 128) // 16] = blkv.reshape(-1, 16).T
        idxw = np.tile(idxw, (8, 1))
        relT = np.ascontiguousarray(slot_rel.reshape(-1, 128).T)
        disS = np.ascontiguousarray(slot_dis.reshape(-1, 128).T)

        nd = np.arange(cfg.NSHpad)
        gidx = np.minimum(c * cfg.NSH + nd, N - 1)
        disP = np.ascontiguousarray(dis[gidx].reshape(-1, 128).T)
        ivdP = np.ascontiguousarray((1.0 / deg[gidx]).reshape(-1, 128).T)

        featT = np.zeros((F, cfg.Npad), dtype=np.float32)
        featT[:, :N] = np.roll(featT_full, -c * cfg.NSH, axis=1)

        in_maps.append({
            "featT": featT, "fcwT": fcwT, "ewT8": ewT8,
            "edgebB": edgebB, "rootB": rootB, "iotaG": iotaG, "ident": ident,
            "efT": efT, "idxw": idxw, "relT": relT, "disS": disS,
            "goff": np.ascontiguousarray(chunk_goff.reshape(1, -1)),
            "disP": disP, "ivdP": ivdP,
        })
    return in_maps


def _cast_maps(in_maps):
    import ml_dtypes
    for m in in_maps:
        m["ewT8"] = m["ewT8"].astype(ml_dtypes.bfloat16)
        m["efT"] = m["efT"].astype(ml_dtypes.bfloat16)
    return in_maps


# ----------------------------------------------------------------- entry ----
def kernel(feat, edge_feat, src, dst, fc_w, edge_w, edge_b, root_emb,
           _trace=False, _cfg=None, **_kw):
    cfg = _cfg or CFG
    feat = np.asarray(feat); edge_feat = np.asarray(edge_feat)
    src = np.asarray(src); dst = np.asarray(dst)
    fc_w = np.asarray(fc_w); edge_w = np.asarray(edge_w)
    edge_b = np.asarray(edge_b); root_emb = np.asarray(root_emb)
    assert feat.shape == (cfg.N, cfg.F) and src.shape == (cfg.E,), \
        (feat.shape, src.shape)
    key = id(cfg) if _cfg is not None else "main"
    if key not in _PROG_CACHE:
        _PROG_CACHE[key] = build_program(cfg)
    nc = _PROG_CACHE[key]
    in_maps = _cast_maps(host_prep(
        cfg, feat, edge_feat, src, dst, fc_w, edge_w, edge_b, root_emb))
    res = bass_utils.run_bass_kernel_spmd(
        nc, in_maps, core_ids=list(range(cfg.cores)), trace=_trace)
    out = np.concatenate(
        [res.results[c]["out"][:cfg.NSH] for c in range(cfg.cores)], axis=0)
    kernel._last_results = res
    return out.astype(np.float32)



# revision 2
# speedup vs baseline: 3.7738x; 3.7738x over previous
"""GCNConv Trainium2 kernel: 8-core SPMD via bass/Tile (v2).

Strategy (dst-range edge sharding; one shared SPMD program, all data per-core):
  - core c owns dst nodes [c*NSH, (c+1)*NSH) and all edges into them
  - table xd = (dis * feat) @ fc_w.T built on device in bf16 (dis = deg^-0.5
    folded into the table; edge_b folded into row 7 of the edge-weight matmul)
  - per-edge chunk (128 edges, grouped per (512-dst-group, src-bucket), fully
    static schedule): dma_gather xd[src] (256B bf16 rows), pw = efd @ ewT9 on
    PE, mpre = gather + pw (DVE), m = relu(mpre) (ACT, bf16), segment-sum via
    matmul with host-precomputed one-hot (streamed from DRAM, bf16) into
    PSUM h^T [feat, 2048-node super-window] at static column offsets
  - node side: out = hT*dis + relu(xd/dis + root)/deg via PE transpose
"""
import sys, math
sys.path.insert(0, "/opt/trn_rl_repo")
import numpy as np

from concourse import bass, bacc, mybir, tile
from concourse import bass_utils

f32 = mybir.dt.float32
bf16 = mybir.dt.bfloat16
i16 = mybir.dt.int16
RELU = mybir.ActivationFunctionType.Relu
ALU = mybir.AluOpType


class Cfg:
    def __init__(self, N=100000, E=1600000, F=128, ED=7, cores=8,
                 grp=512, cap_full=19, cap_last=9, gb=26):
        self.N, self.E, self.F, self.ED, self.cores = N, E, F, ED, cores
        self.NSH = N // cores                    # 12500 nodes per core
        self.GRP = grp                           # one-hot width / group size
        self.SW = 2048                           # psum super-window (4 banks)
        self.n_groups = math.ceil(self.NSH / grp)           # 25
        self.gps = self.SW // grp                # groups per full SW (4)
        self.n_sw = math.ceil(self.n_groups / self.gps)     # 7
        self.last_w = self.NSH - (self.n_groups - 1) * grp  # 212
        self.cap_full, self.cap_last = cap_full, cap_last
        self.n_buckets = 4
        self.bucket_sz = math.ceil(N / self.n_buckets)      # 25000
        self.gb = gb                             # max chunks per gather call
        self.cap = [cap_full] * (self.n_groups - 1) + [cap_last]
        self.chunks_per_b = sum(self.cap)        # per bucket: 24*19+9 = 465
        self.n_chunks = self.n_buckets * self.chunks_per_b  # 1860
        self.slots = self.n_chunks * 128
        self.n_tiles = math.ceil(N / 128)
        self.Npad = self.n_tiles * 128
        self.nsh_tiles = math.ceil(self.NSH / 128)
        self.NSHpad = self.nsh_tiles * 128

    def groups_of_sw(self, s):
        g0 = s * self.gps
        return list(range(g0, min(g0 + self.gps, self.n_groups)))

    def sched(self):
        """Static chunk schedule: list of (sw, bucket, group, start, stop).
        Program order: for sw, for bucket, for group in sw, cap chunks."""
        out = []
        for s in range(self.n_sw):
            gs = self.groups_of_sw(s)
            for b in range(self.n_buckets):
                for g in gs:
                    for k in range(self.cap[g]):
                        start = (b == 0 and k == 0)
                        stop = (b == self.n_buckets - 1 and
                                k == self.cap[g] - 1)
                        out.append((s, b, g, start, stop))
        assert len(out) == self.n_chunks
        return out

    def call_layout(self):
        """Gather-call sizes per (sw, bucket) run: list of chunk counts."""
        out = []
        for s in range(self.n_sw):
            nch = sum(self.cap[g] for g in self.groups_of_sw(s))
            for _b in range(self.n_buckets):
                rem, sizes = nch, []
                while rem > 0:
                    sizes.append(min(self.gb, rem))
                    rem -= sizes[-1]
                out.append(sizes)
        return out


CFG = Cfg()
_PROG_CACHE = {}


# ---------------------------------------------------------------- program ----
def build_program(cfg: Cfg):
    nc = bacc.Bacc("TRN2", target_bir_lowering=False, debug=False,
                   num_devices=cfg.cores)
    F, GRP, SW = cfg.F, cfg.GRP, cfg.SW

    featT_d = nc.dram_tensor("featT", [F, cfg.Npad], f32, kind="ExternalInput")
    fcwT_d = nc.dram_tensor("fcwT", [F, F], f32, kind="ExternalInput")
    ewT9_d = nc.dram_tensor("ewT9", [8, F], bf16, kind="ExternalInput")
    rootB_d = nc.dram_tensor("rootB", [128, F], f32, kind="ExternalInput")
    ident_d = nc.dram_tensor("ident", [128, 128], f32, kind="ExternalInput")
    efT_d = nc.dram_tensor("efT", [8, cfg.slots], bf16, kind="ExternalInput")
    oh_d = nc.dram_tensor("ohT", [128, cfg.n_chunks * GRP], bf16,
                          kind="ExternalInput")
    idx_d = nc.dram_tensor("idxw", [128, cfg.slots // 16], i16,
                           kind="ExternalInput")
    disP_d = nc.dram_tensor("disP", [128, cfg.nsh_tiles], f32,
                            kind="ExternalInput")
    ivdP_d = nc.dram_tensor("ivdP", [128, cfg.nsh_tiles], f32,
                            kind="ExternalInput")
    idisP_d = nc.dram_tensor("idisP", [128, cfg.nsh_tiles], f32,
                             kind="ExternalInput")

    xt_d = nc.dram_tensor("xtab", [cfg.Npad, F], bf16, kind="Internal")
    out_d = nc.dram_tensor("out", [cfg.NSHpad, F], f32, kind="ExternalOutput")

    with tile.TileContext(nc) as tc:
        with tc.tile_pool(name="persist", bufs=1) as pers:
            fcwT = pers.tile([F, F], f32)
            nc.sync.dma_start(out=fcwT[:], in_=fcwT_d.ap())
            ewT9 = pers.tile([8, F], bf16)
            nc.sync.dma_start(out=ewT9[:], in_=ewT9_d.ap())
            rootB = pers.tile([128, F], f32)
            nc.sync.dma_start(out=rootB[:], in_=rootB_d.ap())
            ident = pers.tile([128, 128], f32)
            nc.sync.dma_start(out=ident[:], in_=ident_d.ap())
            idxw = pers.tile([128, cfg.slots // 16], i16)
            nc.sync.dma_start(out=idxw[:], in_=idx_d.ap())
            disP = pers.tile([128, cfg.nsh_tiles], f32)
            nc.sync.dma_start(out=disP[:], in_=disP_d.ap())
            ivdP = pers.tile([128, cfg.nsh_tiles], f32)
            nc.sync.dma_start(out=ivdP[:], in_=ivdP_d.ap())
            idisP = pers.tile([128, cfg.nsh_tiles], f32)
            nc.sync.dma_start(out=idisP[:], in_=idisP_d.ap())
            hT = pers.tile([128, cfg.NSHpad], f32)   # h^T [feat, node]
            nc.vector.memset(hT[:], 0.0)

            # ================= phase 1: xd table =================
            with (
                tc.tile_pool(name="xph", bufs=3) as xph,
                tc.tile_pool(name="xps", bufs=4, space="PSUM") as xps,
            ):
                BLK = 8
                nblk = math.ceil(cfg.n_tiles / BLK)
                for blk in range(nblk):
                    t0 = blk * BLK
                    nt = min(BLK, cfg.n_tiles - t0)
                    ft = xph.tile([F, BLK * 128], f32, tag="ft")
                    nc.sync.dma_start(
                        out=ft[:, :nt * 128],
                        in_=featT_d.ap()[:, t0 * 128:(t0 + nt) * 128])
                    xt = xph.tile([128, BLK, F], bf16, tag="xt")
                    for j in range(nt):
                        px = xps.tile([128, F], f32, tag="px")
                        nc.tensor.matmul(out=px[:],
                                         lhsT=ft[:, j * 128:(j + 1) * 128],
                                         rhs=fcwT[:], start=True, stop=True)
                        nc.vector.tensor_copy(out=xt[:, j, :], in_=px[:])
                    nc.sync.dma_start(
                        out=xt_d.ap()[t0 * 128:(t0 + nt) * 128, :].rearrange(
                            "(b p) f -> p b f", p=128),
                        in_=xt[:, :nt, :])

            # ================= phase 2: edges =================
            sched = cfg.sched()
            calls = cfg.call_layout()
            with (
                tc.tile_pool(name="eph", bufs=2) as eph,
                tc.tile_pool(name="mph", bufs=4) as mph,
                tc.tile_pool(name="hps_pool", bufs=1, space="PSUM") as hps_pool,
                tc.tile_pool(name="wps_pool", bufs=4, space="PSUM") as wps_pool,
            ):
                hps = hps_pool.tile([128, SW], f32)
                ci = 0      # chunk index
                si = 0      # slot index
                run = 0     # (sw, bucket) run index
                for s in range(cfg.n_sw):
                    for b in range(cfg.n_buckets):
                        base = b * cfg.bucket_sz
                        bucket_ap = xt_d.ap()[base:min(base + cfg.bucket_sz,
                                                       cfg.Npad), :]
                        for ncall in calls[run]:
                            nidx = ncall * 128
                            gout = eph.tile([128, cfg.gb, F], bf16, tag="gout")
                            nc.gpsimd.dma_gather(
                                out_ap=gout[:, :ncall, :],
                                in_ap=bucket_ap,
                                idxs_ap=idxw[:, si // 16:(si + nidx) // 16],
                                num_idxs=nidx, num_idxs_reg=nidx, elem_size=F,
                                single_packet=False)
                            ef = eph.tile([8, cfg.gb * 128], bf16, tag="ef")
                            nc.sync.dma_start(
                                out=ef[:, :nidx], in_=efT_d.ap()[:, si:si + nidx])
                            oh = eph.tile([128, cfg.gb * GRP], bf16, tag="oh")
                            nc.sync.dma_start(
                                out=oh[:, :ncall * GRP],
                                in_=oh_d.ap()[:, ci * GRP:(ci + ncall) * GRP])
                            for kk in range(ncall):
                                _s, _b, g, st, sp = sched[ci]
                                assert _s == s and _b == b
                                off = (g - s * cfg.gps) * GRP
                                pw = wps_pool.tile([128, F], f32, tag="pw")
                                nc.tensor.matmul(
                                    out=pw[:],
                                    lhsT=ef[:, kk * 128:(kk + 1) * 128],
                                    rhs=ewT9[:], start=True, stop=True)
                                mpre = mph.tile([128, F], f32, tag="mpre")
                                nc.vector.tensor_add(
                                    out=mpre[:], in0=gout[:, kk, :], in1=pw[:])
                                m = mph.tile([128, F], bf16, tag="m")
                                nc.scalar.activation(
                                    out=m[:], in_=mpre[:], func=RELU)
                                nc.tensor.matmul(
                                    out=hps[:, off:off + GRP],
                                    lhsT=m[:],
                                    rhs=oh[:, kk * GRP:(kk + 1) * GRP],
                                    start=st, stop=sp,
                                    skip_group_check=True)
                                ci += 1
                                si += 128
                        run += 1
                    w = SW if s < cfg.n_sw - 1 else cfg.last_w
                    nc.vector.tensor_add(
                        out=hT[:, s * SW:s * SW + w],
                        in0=hT[:, s * SW:s * SW + w], in1=hps[:, :w])
                assert ci == cfg.n_chunks and si == cfg.slots

            # ================= phase 3: node-side =================
            with (
                tc.tile_pool(name="nph", bufs=3) as nph,
                tc.tile_pool(name="nps", bufs=4, space="PSUM") as nps,
            ):
                NBLK = 8
                for blk in range(math.ceil(cfg.nsh_tiles / NBLK)):
                    t0 = blk * NBLK
                    nt = min(NBLK, cfg.nsh_tiles - t0)
                    xtile = nph.tile([128, NBLK, F], bf16, tag="xtile")
                    nc.sync.dma_start(
                        out=xtile[:, :nt, :],
                        in_=xt_d.ap()[t0 * 128:(t0 + nt) * 128, :].rearrange(
                            "(b p) f -> p b f", p=128))
                    ot = nph.tile([128, NBLK, F], f32, tag="ot")
                    for j in range(nt):
                        t = t0 + j
                        pt = nps.tile([128, F], f32, tag="pt")
                        nc.tensor.transpose(
                            out=pt[:], in_=hT[:, t * 128:(t + 1) * 128],
                            identity=ident[:])
                        s1 = nph.tile([128, F], f32, tag="s1")
                        nc.vector.tensor_scalar_mul(
                            out=s1[:], in0=pt[:], scalar1=disP[:, t:t + 1])
                        x1 = nph.tile([128, F], f32, tag="x1")
                        nc.vector.tensor_scalar_mul(
                            out=x1[:], in0=xtile[:, j, :],
                            scalar1=idisP[:, t:t + 1])
                        t1 = nph.tile([128, F], f32, tag="t1")
                        nc.vector.tensor_add(
                            out=t1[:], in0=x1[:], in1=rootB[:])
                        s2 = nph.tile([128, F], f32, tag="s2")
                        nc.scalar.activation(
                            out=s2[:], in_=t1[:], func=RELU,
                            scale=ivdP[:, t:t + 1])
                        nc.vector.tensor_add(out=ot[:, j, :], in0=s1[:], in1=s2[:])
                    nc.sync.dma_start(
                        out=out_d.ap()[t0 * 128:(t0 + nt) * 128, :].rearrange(
                            "(b p) f -> p b f", p=128),
                        in_=ot[:, :nt, :])
    nc.compile()
    return nc


# ------------------------------------------------------------- host prep ----
def host_prep(cfg: Cfg, feat, edge_feat, src, dst, fc_w, edge_w, edge_b,
              root_emb):
    import ml_dtypes
    bf = ml_dtypes.bfloat16
    N, E, F = cfg.N, cfg.E, cfg.F
    deg = (np.bincount(dst, minlength=N) + 1.0).astype(np.float32)
    dis = deg ** -0.5

    featT_full = np.ascontiguousarray((feat * dis[:, None]).T).astype(np.float32)
    fcwT = np.ascontiguousarray(fc_w.T).astype(np.float32)
    ewT9 = np.zeros((8, F), dtype=np.float32)
    ewT9[:cfg.ED] = edge_w.T
    ewT9[7] = edge_b
    ewT9 = ewT9.astype(bf)
    rootB = np.tile(root_emb[0][None, :], (128, 1)).astype(np.float32)
    ident = np.eye(128, dtype=np.float32)

    core_of = dst // cfg.NSH
    in_maps = []
    for c in range(cfg.cores):
        sel = np.nonzero(core_of == c)[0]
        rsrc = (src[sel] - c * cfg.NSH) % N       # rotated table space
        ed = dst[sel] - c * cfg.NSH
        eb = rsrc // cfg.bucket_sz
        g = ed // cfg.GRP
        s_of_g = np.minimum(g // cfg.gps, cfg.n_sw - 1)
        # schedule-order composite key: (sw, bucket, group)
        comp = (s_of_g * cfg.n_buckets + eb) * cfg.n_groups + g
        order = np.lexsort((ed, comp))
        es, ed, eb, g, comp = (rsrc[order], ed[order], eb[order], g[order],
                               comp[order])
        eid = sel[order]

        slot_src = np.zeros(cfg.slots, dtype=np.int16)
        slot_rel = np.full(cfg.slots, -1, dtype=np.int64)
        slot_eid = np.full(cfg.slots, -1, dtype=np.int64)

        # chunk start offsets in schedule order
        sched = cfg.sched()
        seg_starts = np.searchsorted(
            comp,
            [(s * cfg.n_buckets + b) * cfg.n_groups + gg
             for s in range(cfg.n_sw) for b in range(cfg.n_buckets)
             for gg in cfg.groups_of_sw(s)] + [cfg.n_sw * cfg.n_buckets *
                                               cfg.n_groups])
        ci = 0
        seg = 0
        for s in range(cfg.n_sw):
            gs = cfg.groups_of_sw(s)
            for b in range(cfg.n_buckets):
                for gg in gs:
                    lo, hi = seg_starts[seg], seg_starts[seg + 1]
                    seg += 1
                    nseg = hi - lo
                    need = math.ceil(nseg / 128)
                    if need > cfg.cap[gg]:
                        raise RuntimeError(
                            f"overflow core {c} sw {s} b {b} g {gg}: "
                            f"{need} > {cfg.cap[gg]}")
                    slot0 = ci * 128
                    slot_src[slot0:slot0 + nseg] = (
                        es[lo:hi] - b * cfg.bucket_sz).astype(np.int16)
                    slot_rel[slot0:slot0 + nseg] = ed[lo:hi] - gg * cfg.GRP
                    slot_eid[slot0:slot0 + nseg] = eid[lo:hi]
                    ci += cfg.cap[gg]
        assert ci == cfg.n_chunks

        real = slot_eid >= 0
        efT = np.zeros((8, cfg.slots), dtype=bf)
        sdis = dis[src[slot_eid[real]]].astype(np.float32)
        efT[:cfg.ED, real] = (edge_feat[slot_eid[real]] *
                              sdis[:, None]).T.astype(bf)
        efT[7, real] = sdis.astype(bf)

        # one-hot scatter stream [128, n_chunks*GRP] bf16
        oh = np.zeros((128, cfg.n_chunks * cfg.GRP), dtype=bf)
        slots_idx = np.nonzero(real)[0]
        rows = slots_idx % 128
        cols = (slots_idx // 128) * cfg.GRP + slot_rel[slots_idx]
        oh[rows, cols] = 1.0

        idxw = np.zeros((16, cfg.slots // 16), dtype=np.int16)
        si = 0
        for sizes in cfg.call_layout():
            for nch in sizes:
                blkv = slot_src[si:si + nch * 128]
                idxw[:, si // 16:(si + nch * 128) // 16] = \
                    blkv.reshape(-1, 16).T
                si += nch * 128
        idxw = np.tile(idxw, (8, 1))

        nd = np.arange(cfg.NSHpad)
        gidx = np.minimum(c * cfg.NSH + nd, N - 1)
        disP = np.ascontiguousarray(dis[gidx].reshape(-1, 128).T)
        ivdP = np.ascontiguousarray((1.0 / deg[gidx]).reshape(-1, 128).T)
        idisP = np.ascontiguousarray((1.0 / dis[gidx]).reshape(-1, 128).T)

        featT = np.zeros((F, cfg.Npad), dtype=np.float32)
        featT[:, :N] = np.roll(featT_full, -c * cfg.NSH, axis=1)

        in_maps.append({
            "featT": featT, "fcwT": fcwT, "ewT9": ewT9,
            "rootB": rootB, "ident": ident,
            "efT": efT, "ohT": oh, "idxw": idxw,
            "disP": disP, "ivdP": ivdP, "idisP": idisP,
        })
    return in_maps


# ----------------------------------------------------------------- entry ----
def kernel(feat, edge_feat, src, dst, fc_w, edge_w, edge_b, root_emb,
           _trace=False, _cfg=None, **_kw):
    cfg = _cfg or CFG
    feat = np.asarray(feat); edge_feat = np.asarray(edge_feat)
    src = np.asarray(src); dst = np.asarray(dst)
    fc_w = np.asarray(fc_w); edge_w = np.asarray(edge_w)
    edge_b = np.asarray(edge_b); root_emb = np.asarray(root_emb)
    assert feat.shape == (cfg.N, cfg.F) and src.shape == (cfg.E,), \
        (feat.shape, src.shape)
    key = id(cfg) if _cfg is not None else "main"
    if key not in _PROG_CACHE:
        _PROG_CACHE[key] = build_program(cfg)
    nc = _PROG_CACHE[key]
    in_maps = host_prep(
        cfg, feat, edge_feat, src, dst, fc_w, edge_w, edge_b, root_emb)
    res = bass_utils.run_bass_kernel_spmd(
        nc, in_maps, core_ids=list(range(cfg.cores)), trace=_trace)
    out = np.concatenate(
        [res.results[c]["out"][:cfg.NSH] for c in range(cfg.cores)], axis=0)
    kernel._last_results = res
    return out.astype(np.float32)
